# revision 5
# baseline (speedup 1.0000x reference)
"""ANFIS forward kernel for Trainium2, 8-core data-parallel. v6.

Algebra per row n (see reference):
    l_r = sum_d [2*c*a*x - a*x^2] - k_r;  s_r = exp(l_r)
    G_f = sum_r s_r * Chat[r,f]   (f = (i,o) products + S feature)
    U_o = sum_i xhat_i * G_(i,o);  out = softmax_o(U / (S + eps))

Layout: row n of a core's 16384-row slice -> (p, t) = (n // 128, n % 128).
Host supplies two tensors per core, each loaded in 4 contiguous
quarter-DMAs (one descriptor per partition per quarter):
  xt: [x, x^2] rows (MC, 32)  -- feeds PE transposes; one (128,128)
      transpose covers a whole group (4 tiles x 32 cols).
  xr: [x, 1]   rows (MC, 17)  -- feeds the DVE broadcast multiply.
Quad frontend: 4 groups share one PSUM transpose tile, one ACT
PSUM->SBUF copy, one ACT exp. Per group: M1 (fp32) -> logits,
M2 (f32r, stationary strengths) -> G in PSUM, DVE multiply by xhat,
DVE strided reduce -> U, ACT extracts S+eps. Softmax batched over 32
tiles: reciprocals on DVE, broadcast multiplies on Pool (GpSimd), exp
on ACT. Output stores per meta (4 contiguous DMAs).
"""

import numpy as np

N, D, R, O = 131072, 16, 32, 10
EPS = 1e-8
NCORES = 8
MC = N // NCORES          # rows per core = 16384
TPG = 4                   # tiles (of 128 rows) per group
GROUP = 128 * TPG         # 512 rows per group
NG = MC // GROUP          # 32 groups per core
QUAD = 2                  # groups per frontend batch
NQUAD = NG // QUAD        # 16
META = 8                  # groups per softmax batch
NMETA = NG // META        # 4

DI = D + 1                # 17: x dims + ones
DT = 2 * D                # 32: [x, x^2] row width
F = DI * O                # 170 product features
FS = F + 1                # 171: + strength-sum feature
FPAD = 256                # per-tile feature stride in G4 (bank alignment)
NT = MC // 128            # 128 tiles per core
# input halves (tile_start, n_tiles), packed [xt_h | xr_h] per partition
CH = [(0, 64), (64, 64)]
NTH = NT // 2             # 64 tiles per half
HCOL = NTH * DT + NTH * DI  # 3136 cols per half


def _build_constants(centers, sigmas, coeffs):
    a = 1.0 / (2.0 * sigmas.astype(np.float64) ** 2)          # (R,D)
    c = centers.astype(np.float64)

    # WL4: lhsT for M1. out partition (j,r) = j*32+r.
    # rhs partition ordinal (from the per-group transpose of [x, x^2]
    # rows) is (j, s, d) = j*32 + s*16 + d.
    wl4 = np.zeros((128, 128), np.float64)
    for j in range(TPG):
        for r in range(R):
            pi = j * R + r
            for d in range(D):
                wl4[j * 32 + 0 * 16 + d, pi] = 2.0 * c[r, d] * a[r, d]   # x
                wl4[j * 32 + 1 * 16 + d, pi] = -a[r, d]                  # x^2
    negk = -(c * c * a).sum(axis=1)                            # (R,)
    negk4 = np.tile(negk, TPG).reshape(128, 1)

    # Chat (R, 171): features f = o*17+i (i=16 -> bias row), f=170 -> ones.
    chat = np.zeros((R, FS), np.float64)
    chat[:, :F] = coeffs.astype(np.float64).transpose(0, 2, 1).reshape(
        R, FS - 1)                                              # (R,10*17)
    chat[:, F] = 1.0
    # C2D4 (128, 1024): [(j,r), j'*256+f] = delta_jj' * chat[r,f]
    c2d4 = np.zeros((128, TPG * FPAD), np.float64)
    for j in range(TPG):
        c2d4[j * R:(j + 1) * R, j * FPAD:j * FPAD + FS] = chat
    return (wl4.astype(np.float32), negk4.astype(np.float32),
            c2d4.astype(np.float32))


def _build_bass():
    import concourse.bacc as bacc
    import concourse.mybir as mybir
    from concourse import masks
    from concourse.tile import TileContext

    f32 = mybir.dt.float32
    f32r = mybir.dt.float32r
    AX = mybir.AxisListType
    ALU = mybir.AluOpType
    ACTF = mybir.ActivationFunctionType

    nc = bacc.Bacc("TRN2", target_bir_lowering=False, debug=False)
    xall_d = nc.declare_dram_parameter("xall", [128, 2 * HCOL], f32,
                                       isOutput=False)
    cst_d = nc.declare_dram_parameter("cst", [128, 129], f32, isOutput=False)
    c2d4_d = nc.declare_dram_parameter("c2d4", [128, TPG * FPAD], f32r,
                                       isOutput=False)
    yout = nc.declare_dram_parameter("yout", [MC, O], f32, isOutput=True)

    youtv = yout[:, :].rearrange("(p t) o -> p t o", p=128)

    with TileContext(nc) as tc:
        with (
            tc.tile_pool(name="const", bufs=1) as cpool,
            tc.tile_pool(name="front", bufs=2) as fpool,
            tc.tile_pool(name="work", bufs=3) as wpool,
            tc.tile_pool(name="stage", bufs=2) as spool,
            tc.tile_pool(name="ps_t", bufs=2, space="PSUM") as ps_t,
            tc.tile_pool(name="ps_l", bufs=2, space="PSUM") as ps_l,
            tc.tile_pool(name="ps_g", bufs=2, space="PSUM") as ps_g,
        ):
            ident = cpool.tile([128, 128], f32)
            masks.make_identity(nc, ident[:])

            # first transpose-source quarter goes out first so compute can
            # start as early as possible.
            xall = cpool.tile([128, 2 * HCOL], f32)
            xt_q = [xall[:, 0:NTH * DT],
                    xall[:, HCOL:HCOL + NTH * DT]]
            xr_q = [xall[:, NTH * DT:HCOL],
                    xall[:, HCOL + NTH * DT:2 * HCOL]]
            o_all = cpool.tile([128, NT * O], f32)
            nc.sync.dma_start(out=xall[:, 0:HCOL],
                              in_=xall_d[:, 0:HCOL])
            cst = cpool.tile([128, 129], f32)
            nc.sync.dma_start(out=cst[:], in_=cst_d[:, :])
            negk4 = cst[:, 0:1]
            wl4 = cst[:, 1:129]

            c2d4 = cpool.tile([128, TPG * FPAD], f32r)
            nc.sync.dma_start(out=c2d4[:], in_=c2d4_d[:, :])
            nc.sync.dma_start(out=xall[:, HCOL:2 * HCOL],
                              in_=xall_d[:, HCOL:2 * HCOL])

            for m in range(NMETA):
                u32 = spool.tile([128, META * TPG * O], f32, tag="u32")
                s32 = spool.tile([128, META * TPG], f32, tag="s32")
                e32 = spool.tile([128, META * TPG * O], f32, tag="e32")
                se32 = spool.tile([128, META * TPG], f32, tag="se32")


                for qq in range(META // QUAD):
                    Q = m * (META // QUAD) + qq   # quad index
                    t0 = Q * QUAD * TPG           # first tile of this quad
                    h = next(i for i, (s, n) in enumerate(CH)
                             if s <= t0 < s + n)  # input chunk
                    tb = t0 - CH[h][0]            # tile base within chunk
                    # -- 4 per-group transposes into one PSUM tile ---------
                    xtp = ps_t.tile([128, QUAD * 128], f32, tag="xtp")
                    for k in range(QUAD):
                        nc.tensor.transpose(
                            xtp[:, 128 * k:128 * (k + 1)],
                            xt_q[h][:, DT * (tb + TPG * k):
                                    DT * (tb + TPG * (k + 1))],
                            ident[:])
                    # -- one PSUM->SBUF copy for the whole quad ------------
                    xs = fpool.tile([128, QUAD * 128], f32, tag="xs")
                    nc.scalar.activation(xs[:], xtp[:], ACTF.Copy)
                    # -- M1 x4 into one PSUM bank, one exp -----------------
                    l16 = ps_l.tile([128, QUAD * 128], f32, tag="l16")
                    for k in range(QUAD):
                        nc.tensor.matmul(
                            l16[:, 128 * k:128 * (k + 1)], lhsT=wl4,
                            rhs=xs[:, 128 * k:128 * (k + 1)],
                            start=True, stop=True)
                    sst = fpool.tile([128, QUAD * 128], f32r, tag="sst")
                    nc.scalar.activation(sst[:], l16[:], ACTF.Exp,
                                         bias=negk4, scale=1.0)

                    for k in range(QUAD):
                        q = qq * QUAD + k         # group within meta
                        # -- M2 -------------------------------------------
                        g4 = ps_g.tile([128, TPG * FPAD], f32, tag="g4")
                        nc.tensor.matmul(
                            g4[:, 0:512], lhsT=sst[:, 128 * k:128 * (k + 1)],
                            rhs=c2d4[:, 0:512], start=True, stop=True)
                        nc.tensor.matmul(
                            g4[:, 512:1024],
                            lhsT=sst[:, 128 * k:128 * (k + 1)],
                            rhs=c2d4[:, 512:1024], start=True, stop=True)
                        # -- P = G * xhat (bcast over o) ------------------
                        p4 = wpool.tile([128, TPG * F], f32, tag="p4")
                        p4v = p4[:].rearrange("p (j o i) -> p j o i",
                                              j=TPG, o=O)
                        g4v = g4[:].rearrange("p (j f) -> p j f",
                                              j=TPG)[:, :, 0:F].rearrange(
                            "p j (o i) -> p j o i", o=O)
                        xrv = xr_q[h].rearrange("p (t c) -> p t c", c=DI)
                        xhv = xrv[:, tb + TPG * k:tb + TPG * (k + 1),
                                  :].unsqueeze(2).broadcast_to(
                            [128, TPG, O, DI])
                        nc.vector.tensor_tensor(p4v, g4v, xhv, ALU.mult)
                        # -- S+eps extract (ACT), U = sum_i P (DVE) -------
                        nc.scalar.activation(
                            s32[:, q * TPG:(q + 1) * TPG],
                            g4[:].rearrange("p (j f) -> p j f",
                                            j=TPG)[:, :, F:F + 1].squeeze(2),
                            ACTF.Copy, bias=EPS)
                        nc.vector.tensor_reduce(
                            u32[:, q * TPG * O:(q + 1) * TPG * O].rearrange(
                                "p (j o) -> p j o", j=TPG),
                            p4v,
                            axis=AX.X, op=ALU.add)

                # -- batched normalize + softmax over 32 tiles -------------
                nc.vector.reciprocal(s32[:], s32[:])
                u32v = u32[:].rearrange("p (g o) -> p g o", o=O)
                s32b = s32[:].unsqueeze(2).broadcast_to(
                    [128, META * TPG, O])
                eng_tt = nc.vector if m == NMETA - 1 else nc.gpsimd
                eng_tt.tensor_tensor(u32v, u32v, s32b, ALU.mult)
                nc.scalar.activation(e32[:], u32[:], ACTF.Exp)
                nc.vector.tensor_reduce(
                    se32[:], e32[:].rearrange("p (g o) -> p g o", o=O),
                    axis=AX.X, op=ALU.add)
                nc.vector.reciprocal(se32[:], se32[:])
                se32b = se32[:].unsqueeze(2).broadcast_to(
                    [128, META * TPG, O])
                eng_tt.tensor_tensor(
                    o_all[:, m * META * TPG * O:(m + 1) * META * TPG * O
                          ].rearrange("p (g o) -> p g o", o=O),
                    e32[:].rearrange("p (g o) -> p g o", o=O),
                    se32b, ALU.mult)

            # -- one contiguous store for all 16384 rows -------------------
            nc.sync.dma_start(
                out=youtv[:, :, :],
                in_=o_all[:].rearrange("p (t o) -> p t o", o=O))
    nc.compile()
    return nc


def _pack(xt2c, xaugc):
    """(128, 2*HCOL): per partition [xt_h0 | xr_h0 | xt_h1 | xr_h1]."""
    xtr = xt2c.reshape(128, NT, DT)
    xrr = xaugc.reshape(128, NT, DI)
    parts = []
    for hh in range(2):
        sl = slice(hh * NTH, (hh + 1) * NTH)
        parts.append(xtr[:, sl].reshape(128, -1))
        parts.append(xrr[:, sl].reshape(128, -1))
    return np.ascontiguousarray(np.concatenate(parts, axis=1))


_NC_CACHE = None


def kernel(X, centers, sigmas, coeffs):
    global _NC_CACHE
    from concourse import bass_utils

    X = np.asarray(X, np.float32)
    wl4, negk4, c2d4 = _build_constants(
        np.asarray(centers, np.float32),
        np.asarray(sigmas, np.float32),
        np.asarray(coeffs, np.float32))
    cst = np.concatenate([negk4, wl4], axis=1)

    xaug = np.ones((N, DI), np.float32)
    xaug[:, 0:D] = X
    xt2 = np.empty((N, DT), np.float32)
    xt2[:, 0:D] = X
    xt2[:, D:DT] = X * X

    if _NC_CACHE is None:
        _NC_CACHE = _build_bass()
    nc = _NC_CACHE

    in_maps = []
    for c in range(NCORES):
        in_maps.append({
            "xall": _pack(xt2[c * MC:(c + 1) * MC],
                          xaug[c * MC:(c + 1) * MC]),
            "cst": cst, "c2d4": c2d4,
        })
    res = bass_utils.run_bass_kernel_spmd(nc, in_maps, list(range(NCORES)))
    return np.concatenate([r["yout"] for r in res.results], axis=0)


# revision 6
# speedup vs baseline: 1.0380x; 1.0380x over previous
"""ANFIS forward kernel for Trainium2, 8-core data-parallel. v6.

Algebra per row n (see reference):
    l_r = sum_d [2*c*a*x - a*x^2] - k_r;  s_r = exp(l_r)
    G_f = sum_r s_r * Chat[r,f]   (f = (i,o) products + S feature)
    U_o = sum_i xhat_i * G_(i,o);  out = softmax_o(U / (S + eps))

Layout: row n of a core's 16384-row slice -> (p, t) = (n // 128, n % 128).
Host supplies two tensors per core, each loaded in 4 contiguous
quarter-DMAs (one descriptor per partition per quarter):
  xt: [x, x^2] rows (MC, 32)  -- feeds PE transposes; one (128,128)
      transpose covers a whole group (4 tiles x 32 cols).
  xr: [x, 1]   rows (MC, 17)  -- feeds the DVE broadcast multiply.
Quad frontend: 4 groups share one PSUM transpose tile, one ACT
PSUM->SBUF copy, one ACT exp. Per group: M1 (fp32) -> logits,
M2 (f32r, stationary strengths) -> G in PSUM, DVE multiply by xhat,
DVE strided reduce -> U, ACT extracts S+eps. Softmax batched over 32
tiles: reciprocals on DVE, broadcast multiplies on Pool (GpSimd), exp
on ACT. Output stores per meta (4 contiguous DMAs).
"""

import numpy as np

N, D, R, O = 131072, 16, 32, 10
EPS = 1e-8
NCORES = 8
MC = N // NCORES          # rows per core = 16384
TPG = 4                   # tiles (of 128 rows) per group
GROUP = 128 * TPG         # 512 rows per group
NG = MC // GROUP          # 32 groups per core
QUAD = 2                  # groups per frontend batch
NQUAD = NG // QUAD        # 16
META = 8                  # groups per softmax batch
NMETA = NG // META        # 4

DI = D + 1                # 17: x dims + ones
DT = 2 * D                # 32: [x, x^2] row width
F = DI * O                # 170 product features
FS = F + 1                # 171: + strength-sum feature
FPAD = 256                # per-tile feature stride in G4 (bank alignment)
NT = MC // 128            # 128 tiles per core
# input chunks (tile_start, n_tiles), packed [xt | xr] per partition per
# chunk; small first chunk -> early compute start, still one DMA each.
CH = [(0, 16), (16, 112)]
RW = DT + DI              # 49 cols per tile in the packed tensor
CB = [s * RW for s, n in CH]  # chunk base columns


def _build_constants(centers, sigmas, coeffs):
    a = 1.0 / (2.0 * sigmas.astype(np.float64) ** 2)          # (R,D)
    c = centers.astype(np.float64)

    # WL4: lhsT for M1. out partition (j,r) = j*32+r.
    # rhs partition ordinal (from the per-group transpose of [x, x^2]
    # rows) is (j, s, d) = j*32 + s*16 + d.
    wl4 = np.zeros((128, 128), np.float64)
    for j in range(TPG):
        for r in range(R):
            pi = j * R + r
            for d in range(D):
                wl4[j * 32 + 0 * 16 + d, pi] = 2.0 * c[r, d] * a[r, d]   # x
                wl4[j * 32 + 1 * 16 + d, pi] = -a[r, d]                  # x^2
    negk = -(c * c * a).sum(axis=1)                            # (R,)
    negk4 = np.tile(negk, TPG).reshape(128, 1)

    # Chat (R, 171): features f = o*17+i (i=16 -> bias row), f=170 -> ones.
    chat = np.zeros((R, FS), np.float64)
    chat[:, :F] = coeffs.astype(np.float64).transpose(0, 2, 1).reshape(
        R, FS - 1)                                              # (R,10*17)
    chat[:, F] = 1.0
    # C2D4 (128, 1024): [(j,r), j'*256+f] = delta_jj' * chat[r,f]
    c2d4 = np.zeros((128, TPG * FPAD), np.float64)
    for j in range(TPG):
        c2d4[j * R:(j + 1) * R, j * FPAD:j * FPAD + FS] = chat
    return (wl4.astype(np.float32), negk4.astype(np.float32),
            c2d4.astype(np.float32))


def _build_bass():
    import concourse.bacc as bacc
    import concourse.mybir as mybir
    from concourse import masks
    from concourse.tile import TileContext

    f32 = mybir.dt.float32
    f32r = mybir.dt.float32r
    AX = mybir.AxisListType
    ALU = mybir.AluOpType
    ACTF = mybir.ActivationFunctionType

    nc = bacc.Bacc("TRN2", target_bir_lowering=False, debug=False)
    xall_d = nc.declare_dram_parameter("xall", [128, NT * RW], f32,
                                       isOutput=False)
    cst_d = nc.declare_dram_parameter("cst", [128, 129], f32, isOutput=False)
    c2d4_d = nc.declare_dram_parameter("c2d4", [128, TPG * FPAD], f32r,
                                       isOutput=False)
    yout = nc.declare_dram_parameter("yout", [MC, O], f32, isOutput=True)

    youtv = yout[:, :].rearrange("(p t) o -> p t o", p=128)

    with TileContext(nc) as tc:
        with (
            tc.tile_pool(name="const", bufs=1) as cpool,
            tc.tile_pool(name="front", bufs=2) as fpool,
            tc.tile_pool(name="work", bufs=3) as wpool,
            tc.tile_pool(name="stage", bufs=2) as spool,
            tc.tile_pool(name="ps_t", bufs=2, space="PSUM") as ps_t,
            tc.tile_pool(name="ps_l", bufs=2, space="PSUM") as ps_l,
            tc.tile_pool(name="ps_g", bufs=2, space="PSUM") as ps_g,
        ):
            ident = cpool.tile([128, 128], f32)
            masks.make_identity(nc, ident[:])

            # first transpose-source quarter goes out first so compute can
            # start as early as possible.
            xall = cpool.tile([128, NT * RW], f32)
            xt_q = [xall[:, CB[i]:CB[i] + CH[i][1] * DT]
                    for i in range(len(CH))]
            xr_q = [xall[:, CB[i] + CH[i][1] * DT:CB[i] + CH[i][1] * RW]
                    for i in range(len(CH))]
            o_all = cpool.tile([128, NT * O], f32)
            nc.sync.dma_start(out=xall[:, 0:CB[1]],
                              in_=xall_d[:, 0:CB[1]])
            cst = cpool.tile([128, 129], f32)
            nc.sync.dma_start(out=cst[:], in_=cst_d[:, :])
            negk4 = cst[:, 0:1]
            wl4 = cst[:, 1:129]

            c2d4 = cpool.tile([128, TPG * FPAD], f32r)
            nc.sync.dma_start(out=c2d4[:], in_=c2d4_d[:, :])
            nc.sync.dma_start(out=xall[:, CB[1]:NT * RW],
                              in_=xall_d[:, CB[1]:NT * RW])

            for m in range(NMETA):
                u32 = spool.tile([128, META * TPG * O], f32, tag="u32")
                s32 = spool.tile([128, META * TPG], f32, tag="s32")
                e32 = spool.tile([128, META * TPG * O], f32, tag="e32")
                se32 = spool.tile([128, META * TPG], f32, tag="se32")


                for qq in range(META // QUAD):
                    Q = m * (META // QUAD) + qq   # quad index
                    t0 = Q * QUAD * TPG           # first tile of this quad
                    h = next(i for i, (s, n) in enumerate(CH)
                             if s <= t0 < s + n)  # input chunk
                    tb = t0 - CH[h][0]            # tile base within chunk
                    # -- 4 per-group transposes into one PSUM tile ---------
                    xtp = ps_t.tile([128, QUAD * 128], f32, tag="xtp")
                    for k in range(QUAD):
                        nc.tensor.transpose(
                            xtp[:, 128 * k:128 * (k + 1)],
                            xt_q[h][:, DT * (tb + TPG * k):
                                    DT * (tb + TPG * (k + 1))],
                            ident[:])
                    # -- one PSUM->SBUF copy for the whole quad ------------
                    xs = fpool.tile([128, QUAD * 128], f32, tag="xs")
                    nc.scalar.activation(xs[:], xtp[:], ACTF.Copy)
                    # -- M1 x4 into one PSUM bank, one exp -----------------
                    l16 = ps_l.tile([128, QUAD * 128], f32, tag="l16")
                    for k in range(QUAD):
                        nc.tensor.matmul(
                            l16[:, 128 * k:128 * (k + 1)], lhsT=wl4,
                            rhs=xs[:, 128 * k:128 * (k + 1)],
                            start=True, stop=True)
                    sst = fpool.tile([128, QUAD * 128], f32r, tag="sst")
                    nc.scalar.activation(sst[:], l16[:], ACTF.Exp,
                                         bias=negk4, scale=1.0)

                    for k in range(QUAD):
                        q = qq * QUAD + k         # group within meta
                        # -- M2 -------------------------------------------
                        g4 = ps_g.tile([128, TPG * FPAD], f32, tag="g4")
                        nc.tensor.matmul(
                            g4[:, 0:512], lhsT=sst[:, 128 * k:128 * (k + 1)],
                            rhs=c2d4[:, 0:512], start=True, stop=True)
                        nc.tensor.matmul(
                            g4[:, 512:1024],
                            lhsT=sst[:, 128 * k:128 * (k + 1)],
                            rhs=c2d4[:, 512:1024], start=True, stop=True)
                        # -- P = G * xhat (bcast over o) ------------------
                        p4 = wpool.tile([128, TPG * F], f32, tag="p4")
                        p4v = p4[:].rearrange("p (j o i) -> p j o i",
                                              j=TPG, o=O)
                        g4v = g4[:].rearrange("p (j f) -> p j f",
                                              j=TPG)[:, :, 0:F].rearrange(
                            "p j (o i) -> p j o i", o=O)
                        xrv = xr_q[h].rearrange("p (t c) -> p t c", c=DI)
                        xhv = xrv[:, tb + TPG * k:tb + TPG * (k + 1),
                                  :].unsqueeze(2).broadcast_to(
                            [128, TPG, O, DI])
                        nc.vector.tensor_tensor(p4v, g4v, xhv, ALU.mult)
                        # -- S+eps extract (ACT), U = sum_i P (DVE) -------
                        nc.scalar.activation(
                            s32[:, q * TPG:(q + 1) * TPG],
                            g4[:].rearrange("p (j f) -> p j f",
                                            j=TPG)[:, :, F:F + 1].squeeze(2),
                            ACTF.Copy, bias=EPS)
                        nc.vector.tensor_reduce(
                            u32[:, q * TPG * O:(q + 1) * TPG * O].rearrange(
                                "p (j o) -> p j o", j=TPG),
                            p4v,
                            axis=AX.X, op=ALU.add)

                # -- batched normalize + softmax over 32 tiles -------------
                nc.vector.reciprocal(s32[:], s32[:])
                u32v = u32[:].rearrange("p (g o) -> p g o", o=O)
                s32b = s32[:].unsqueeze(2).broadcast_to(
                    [128, META * TPG, O])
                eng_tt = nc.vector if m == NMETA - 1 else nc.gpsimd
                eng_tt.tensor_tensor(u32v, u32v, s32b, ALU.mult)
                nc.scalar.activation(e32[:], u32[:], ACTF.Exp)
                nc.vector.tensor_reduce(
                    se32[:], e32[:].rearrange("p (g o) -> p g o", o=O),
                    axis=AX.X, op=ALU.add)
                nc.vector.reciprocal(se32[:], se32[:])
                se32b = se32[:].unsqueeze(2).broadcast_to(
                    [128, META * TPG, O])
                eng_tt.tensor_tensor(
                    o_all[:, m * META * TPG * O:(m + 1) * META * TPG * O
                          ].rearrange("p (g o) -> p g o", o=O),
                    e32[:].rearrange("p (g o) -> p g o", o=O),
                    se32b, ALU.mult)

            # -- one contiguous store for all 16384 rows -------------------
            nc.sync.dma_start(
                out=youtv[:, :, :],
                in_=o_all[:].rearrange("p (t o) -> p t o", o=O))
    nc.compile()
    return nc


def _pack(xt2c, xaugc):
    """(128, NT*RW): per partition, per chunk [xt-tiles | xr-tiles]."""
    xtr = xt2c.reshape(128, NT, DT)
    xrr = xaugc.reshape(128, NT, DI)
    parts = []
    for s, n in CH:
        parts.append(xtr[:, s:s + n].reshape(128, -1))
        parts.append(xrr[:, s:s + n].reshape(128, -1))
    return np.ascontiguousarray(np.concatenate(parts, axis=1))


_NC_CACHE = None


def kernel(X, centers, sigmas, coeffs):
    global _NC_CACHE
    from concourse import bass_utils

    X = np.asarray(X, np.float32)
    wl4, negk4, c2d4 = _build_constants(
        np.asarray(centers, np.float32),
        np.asarray(sigmas, np.float32),
        np.asarray(coeffs, np.float32))
    cst = np.concatenate([negk4, wl4], axis=1)

    xaug = np.ones((N, DI), np.float32)
    xaug[:, 0:D] = X
    xt2 = np.empty((N, DT), np.float32)
    xt2[:, 0:D] = X
    xt2[:, D:DT] = X * X

    if _NC_CACHE is None:
        _NC_CACHE = _build_bass()
    nc = _NC_CACHE

    in_maps = []
    for c in range(NCORES):
        in_maps.append({
            "xall": _pack(xt2[c * MC:(c + 1) * MC],
                          xaug[c * MC:(c + 1) * MC]),
            "cst": cst, "c2d4": c2d4,
        })
    res = bass_utils.run_bass_kernel_spmd(nc, in_maps, list(range(NCORES)))
    return np.concatenate([r["yout"] for r in res.results], axis=0)


# revision 8
# speedup vs baseline: 1.0505x; 1.0120x over previous
"""ANFIS forward kernel for Trainium2, 8-core data-parallel. v6.

Algebra per row n (see reference):
    l_r = sum_d [2*c*a*x - a*x^2] - k_r;  s_r = exp(l_r)
    G_f = sum_r s_r * Chat[r,f]   (f = (i,o) products + S feature)
    U_o = sum_i xhat_i * G_(i,o);  out = softmax_o(U / (S + eps))

Layout: row n of a core's 16384-row slice -> (p, t) = (n // 128, n % 128).
Host supplies two tensors per core, each loaded in 4 contiguous
quarter-DMAs (one descriptor per partition per quarter):
  xt: [x, x^2] rows (MC, 32)  -- feeds PE transposes; one (128,128)
      transpose covers a whole group (4 tiles x 32 cols).
  xr: [x, 1]   rows (MC, 17)  -- feeds the DVE broadcast multiply.
Quad frontend: 4 groups share one PSUM transpose tile, one ACT
PSUM->SBUF copy, one ACT exp. Per group: M1 (fp32) -> logits,
M2 (f32r, stationary strengths) -> G in PSUM, DVE multiply by xhat,
DVE strided reduce -> U, ACT extracts S+eps. Softmax batched over 32
tiles: reciprocals on DVE, broadcast multiplies on Pool (GpSimd), exp
on ACT. Output stores per meta (4 contiguous DMAs).
"""

import numpy as np

N, D, R, O = 131072, 16, 32, 10
EPS = 1e-8
NCORES = 8
MC = N // NCORES          # rows per core = 16384
TPG = 4                   # tiles (of 128 rows) per group
GROUP = 128 * TPG         # 512 rows per group
NG = MC // GROUP          # 32 groups per core
QUAD = 2                  # groups per frontend batch
NQUAD = NG // QUAD        # 16
META = 8                  # groups per softmax batch
NMETA = NG // META        # 4

DI = D + 1                # 17: x dims + ones
DT = 2 * D                # 32: [x, x^2] row width
F = DI * O                # 170 product features
FS = F + 1                # 171: + strength-sum feature
FPAD = 256                # per-tile feature stride in G4 (bank alignment)
NT = MC // 128            # 128 tiles per core
# input chunks (tile_start, n_tiles), packed [xt | xr] per partition per
# chunk; small first chunk -> early compute start, still one DMA each.
CH = [(0, 24), (24, 104)]
RW = DT + DI              # 49 cols per tile in the packed tensor
CB = [s * RW for s, n in CH]  # chunk base columns


def _build_constants(centers, sigmas, coeffs):
    a = 1.0 / (2.0 * sigmas.astype(np.float64) ** 2)          # (R,D)
    c = centers.astype(np.float64)

    # WL4: lhsT for M1. out partition (j,r) = j*32+r.
    # rhs partition ordinal (from the per-group transpose of [x, x^2]
    # rows) is (j, s, d) = j*32 + s*16 + d.
    wl4 = np.zeros((128, 128), np.float64)
    for j in range(TPG):
        for r in range(R):
            pi = j * R + r
            for d in range(D):
                wl4[j * 32 + 0 * 16 + d, pi] = 2.0 * c[r, d] * a[r, d]   # x
                wl4[j * 32 + 1 * 16 + d, pi] = -a[r, d]                  # x^2
    negk = -(c * c * a).sum(axis=1)                            # (R,)
    negk4 = np.tile(negk, TPG).reshape(128, 1)

    # Chat (R, 171): features f = o*17+i (i=16 -> bias row), f=170 -> ones.
    chat = np.zeros((R, FS), np.float64)
    chat[:, :F] = coeffs.astype(np.float64).transpose(0, 2, 1).reshape(
        R, FS - 1)                                              # (R,10*17)
    chat[:, F] = 1.0
    # C2D4 (128, 1024): [(j,r), j'*256+f] = delta_jj' * chat[r,f]
    c2d4 = np.zeros((128, TPG * FPAD), np.float64)
    for j in range(TPG):
        c2d4[j * R:(j + 1) * R, j * FPAD:j * FPAD + FS] = chat
    return (wl4.astype(np.float32), negk4.astype(np.float32),
            c2d4.astype(np.float32))


def _build_bass():
    import concourse.bacc as bacc
    import concourse.mybir as mybir
    from concourse import masks
    from concourse.tile import TileContext

    f32 = mybir.dt.float32
    f32r = mybir.dt.float32r
    AX = mybir.AxisListType
    ALU = mybir.AluOpType
    ACTF = mybir.ActivationFunctionType

    nc = bacc.Bacc("TRN2", target_bir_lowering=False, debug=False)
    xall_d = nc.declare_dram_parameter("xall", [128, NT * RW], f32,
                                       isOutput=False)
    cst_d = nc.declare_dram_parameter("cst", [128, 129], f32, isOutput=False)
    c2d4_d = nc.declare_dram_parameter("c2d4", [128, TPG * FPAD], f32r,
                                       isOutput=False)
    yout = nc.declare_dram_parameter("yout", [MC, O], f32, isOutput=True)

    youtv = yout[:, :].rearrange("(p t) o -> p t o", p=128)

    with TileContext(nc) as tc:
        with (
            tc.tile_pool(name="const", bufs=1) as cpool,
            tc.tile_pool(name="front", bufs=2) as fpool,
            tc.tile_pool(name="work", bufs=3) as wpool,
            tc.tile_pool(name="stage", bufs=3) as spool,
            tc.tile_pool(name="ps_t", bufs=2, space="PSUM") as ps_t,
            tc.tile_pool(name="ps_l", bufs=2, space="PSUM") as ps_l,
            tc.tile_pool(name="ps_g", bufs=2, space="PSUM") as ps_g,
        ):
            ident = cpool.tile([128, 128], f32)
            masks.make_identity(nc, ident[:])

            # first transpose-source quarter goes out first so compute can
            # start as early as possible.
            xall = cpool.tile([128, NT * RW], f32)
            xt_q = [xall[:, CB[i]:CB[i] + CH[i][1] * DT]
                    for i in range(len(CH))]
            xr_q = [xall[:, CB[i] + CH[i][1] * DT:CB[i] + CH[i][1] * RW]
                    for i in range(len(CH))]
            o_all = cpool.tile([128, NT * O], f32)
            nc.sync.dma_start(out=xall[:, 0:CB[1]],
                              in_=xall_d[:, 0:CB[1]])
            cst = cpool.tile([128, 129], f32)
            nc.sync.dma_start(out=cst[:], in_=cst_d[:, :])
            negk4 = cst[:, 0:1]
            wl4 = cst[:, 1:129]

            c2d4 = cpool.tile([128, TPG * FPAD], f32r)
            nc.sync.dma_start(out=c2d4[:], in_=c2d4_d[:, :])
            nc.sync.dma_start(out=xall[:, CB[1]:NT * RW],
                              in_=xall_d[:, CB[1]:NT * RW])

            for m in range(NMETA):
                u32 = spool.tile([128, META * TPG * O], f32, tag="u32")
                s32 = spool.tile([128, META * TPG], f32, tag="s32")
                e32 = spool.tile([128, META * TPG * O], f32, tag="e32")
                se32 = spool.tile([128, META * TPG], f32, tag="se32")


                for qq in range(META // QUAD):
                    Q = m * (META // QUAD) + qq   # quad index
                    t0 = Q * QUAD * TPG           # first tile of this quad
                    h = next(i for i, (s, n) in enumerate(CH)
                             if s <= t0 < s + n)  # input chunk
                    tb = t0 - CH[h][0]            # tile base within chunk
                    # -- 4 per-group transposes into one PSUM tile ---------
                    xtp = ps_t.tile([128, QUAD * 128], f32, tag="xtp")
                    for k in range(QUAD):
                        nc.tensor.transpose(
                            xtp[:, 128 * k:128 * (k + 1)],
                            xt_q[h][:, DT * (tb + TPG * k):
                                    DT * (tb + TPG * (k + 1))],
                            ident[:])
                    # -- one PSUM->SBUF copy for the whole quad ------------
                    xs = fpool.tile([128, QUAD * 128], f32, tag="xs")
                    nc.scalar.activation(xs[:], xtp[:], ACTF.Copy)
                    # -- M1 x4 into one PSUM bank, one exp -----------------
                    l16 = ps_l.tile([128, QUAD * 128], f32, tag="l16")
                    for k in range(QUAD):
                        nc.tensor.matmul(
                            l16[:, 128 * k:128 * (k + 1)], lhsT=wl4,
                            rhs=xs[:, 128 * k:128 * (k + 1)],
                            start=True, stop=True)
                    sst = fpool.tile([128, QUAD * 128], f32r, tag="sst")
                    nc.scalar.activation(sst[:], l16[:], ACTF.Exp,
                                         bias=negk4, scale=1.0)

                    for k in range(QUAD):
                        q = qq * QUAD + k         # group within meta
                        # -- M2 -------------------------------------------
                        g4 = ps_g.tile([128, TPG * FPAD], f32, tag="g4")
                        nc.tensor.matmul(
                            g4[:, 0:512], lhsT=sst[:, 128 * k:128 * (k + 1)],
                            rhs=c2d4[:, 0:512], start=True, stop=True)
                        nc.tensor.matmul(
                            g4[:, 512:1024],
                            lhsT=sst[:, 128 * k:128 * (k + 1)],
                            rhs=c2d4[:, 512:1024], start=True, stop=True)
                        # -- P = G * xhat (bcast over o) ------------------
                        p4 = wpool.tile([128, TPG * F], f32, tag="p4")
                        p4v = p4[:].rearrange("p (j o i) -> p j o i",
                                              j=TPG, o=O)
                        g4v = g4[:].rearrange("p (j f) -> p j f",
                                              j=TPG)[:, :, 0:F].rearrange(
                            "p j (o i) -> p j o i", o=O)
                        xrv = xr_q[h].rearrange("p (t c) -> p t c", c=DI)
                        xhv = xrv[:, tb + TPG * k:tb + TPG * (k + 1),
                                  :].unsqueeze(2).broadcast_to(
                            [128, TPG, O, DI])
                        nc.vector.tensor_tensor(p4v, g4v, xhv, ALU.mult)
                        # -- S+eps extract (ACT), U = sum_i P (DVE) -------
                        nc.scalar.activation(
                            s32[:, q * TPG:(q + 1) * TPG],
                            g4[:].rearrange("p (j f) -> p j f",
                                            j=TPG)[:, :, F:F + 1].squeeze(2),
                            ACTF.Copy, bias=EPS)
                        nc.vector.tensor_reduce(
                            u32[:, q * TPG * O:(q + 1) * TPG * O].rearrange(
                                "p (j o) -> p j o", j=TPG),
                            p4v,
                            axis=AX.X, op=ALU.add)

                # -- batched normalize + softmax over 32 tiles -------------
                nc.vector.reciprocal(s32[:], s32[:])
                u32v = u32[:].rearrange("p (g o) -> p g o", o=O)
                s32b = s32[:].unsqueeze(2).broadcast_to(
                    [128, META * TPG, O])
                eng_tt = nc.vector if m == NMETA - 1 else nc.gpsimd
                eng_tt.tensor_tensor(u32v, u32v, s32b, ALU.mult)
                nc.scalar.activation(e32[:], u32[:], ACTF.Exp)
                nc.vector.tensor_reduce(
                    se32[:], e32[:].rearrange("p (g o) -> p g o", o=O),
                    axis=AX.X, op=ALU.add)
                nc.vector.reciprocal(se32[:], se32[:])
                se32b = se32[:].unsqueeze(2).broadcast_to(
                    [128, META * TPG, O])
                eng_tt.tensor_tensor(
                    o_all[:, m * META * TPG * O:(m + 1) * META * TPG * O
                          ].rearrange("p (g o) -> p g o", o=O),
                    e32[:].rearrange("p (g o) -> p g o", o=O),
                    se32b, ALU.mult)

            # -- one contiguous store for all 16384 rows -------------------
            nc.sync.dma_start(
                out=youtv[:, :, :],
                in_=o_all[:].rearrange("p (t o) -> p t o", o=O))
    nc.compile()
    return nc


def _pack(xt2c, xaugc):
    """(128, NT*RW): per partition, per chunk [xt-tiles | xr-tiles]."""
    xtr = xt2c.reshape(128, NT, DT)
    xrr = xaugc.reshape(128, NT, DI)
    parts = []
    for s, n in CH:
        parts.append(xtr[:, s:s + n].reshape(128, -1))
        parts.append(xrr[:, s:s + n].reshape(128, -1))
    return np.ascontiguousarray(np.concatenate(parts, axis=1))


_NC_CACHE = None


def kernel(X, centers, sigmas, coeffs):
    global _NC_CACHE
    from concourse import bass_utils

    X = np.asarray(X, np.float32)
    wl4, negk4, c2d4 = _build_constants(
        np.asarray(centers, np.float32),
        np.asarray(sigmas, np.float32),
        np.asarray(coeffs, np.float32))
    cst = np.concatenate([negk4, wl4], axis=1)

    xaug = np.ones((N, DI), np.float32)
    xaug[:, 0:D] = X
    xt2 = np.empty((N, DT), np.float32)
    xt2[:, 0:D] = X
    xt2[:, D:DT] = X * X

    if _NC_CACHE is None:
        _NC_CACHE = _build_bass()
    nc = _NC_CACHE

    in_maps = []
    for c in range(NCORES):
        in_maps.append({
            "xall": _pack(xt2[c * MC:(c + 1) * MC],
                          xaug[c * MC:(c + 1) * MC]),
            "cst": cst, "c2d4": c2d4,
        })
    res = bass_utils.run_bass_kernel_spmd(nc, in_maps, list(range(NCORES)))
    return np.concatenate([r["yout"] for r in res.results], axis=0)
